# revision 1
# baseline (speedup 1.0000x reference)
"""MLA attention (DeepSeek-style) Trainium2 Bass kernel, 8-core SPMD.

Sharding: core c handles batch b = c//4 and head-group g = c%4 (4 of 16 heads).
Down-projections are replicated per batch; up-projections / attention / o-proj
are head-parallel. Host sums the 4 partial o-projections per batch.

Device dataflow (per core, transposed-activation layout, S processed in 4
chunks of 512):
  xT (host-transposed, bf16) -> q_latT/kv_latT (bf16 matmuls -> f32r latents)
  -> per-head qT/kT [128=HD, S] bf16 tiles assembled from PSUM (nope rows
  0:64, rope rows 64:128; q uses host-concatenated Wq_up|Wq_rope columns),
  RoPE via host-baked cos/sin tables -> causal flash attention per head:
  scoresT [j,i] matmuls, exp on ScalarE (scale fused), unnormalized attnout +
  ones-matmul row sums, normalize by broadcast reciprocal -> o-projection ->
  partial [S, D] f32 out.
"""

import numpy as np
import ml_dtypes

import concourse.bacc as bacc
import concourse.mybir as mybir
import concourse.tile as tile
from concourse.bass_utils import run_bass_kernel_spmd

F32 = mybir.dt.float32
F32R = mybir.dt.float32r
BF16 = mybir.dt.bfloat16

B, S, D = 2, 2048, 2048
H, HD = 16, 128
RD, ND = 64, 64
KVR, QR = 512, 1024
BASE = 10000.0
HLOC = 4                 # heads per core
CHUNK = 512
NCHUNK = S // CHUNK      # 4
P = 128
SCALE = HD ** -0.5

_BF16 = ml_dtypes.bfloat16


def _build():
    nc = bacc.Bacc("TRN2", target_bir_lowering=False, debug=False)

    xT = nc.dram_tensor("xT", [D, S], BF16, kind="ExternalInput").ap()
    wqd = nc.dram_tensor("wqd", [D, QR], BF16, kind="ExternalInput").ap()
    wkvd = nc.dram_tensor("wkvd", [D, KVR], BF16, kind="ExternalInput").ap()
    wkr = nc.dram_tensor("wkr", [D, HLOC * RD], BF16, kind="ExternalInput").ap()
    wqcat = nc.dram_tensor("wqcat", [QR, HLOC * HD], F32, kind="ExternalInput").ap()
    wkup = nc.dram_tensor("wkup", [KVR, HLOC * ND], F32, kind="ExternalInput").ap()
    wvup = nc.dram_tensor("wvup", [KVR, HLOC * HD], F32, kind="ExternalInput").ap()
    wo = nc.dram_tensor("wo", [HLOC * HD, D], BF16, kind="ExternalInput").ap()
    cosr = nc.dram_tensor("cosr", [P, S], F32, kind="ExternalInput").ap()
    sinr = nc.dram_tensor("sinr", [P, S], F32, kind="ExternalInput").ap()
    maskd = nc.dram_tensor("maskd", [P, 4 * CHUNK], BF16, kind="ExternalInput").ap()
    o_part = nc.dram_tensor("o_part", [S, D], F32, kind="ExternalOutput").ap()

    xT_r = xT.rearrange("(dt p) s -> p dt s", p=P)          # [128, 16, S]
    wqd_r = wqd.rearrange("(dt p) q -> p dt q", p=P)        # [128, 16, 1024]
    wkvd_r = wkvd.rearrange("(dt p) q -> p dt q", p=P)      # [128, 16, 512]
    wkr_r = wkr.rearrange("(dt p) q -> p dt q", p=P)        # [128, 16, 256]
    wqcat_r = wqcat.rearrange("(qt p) c -> p qt c", p=P)    # [128, 8, 512]
    wkup_r = wkup.rearrange("(kt p) c -> p kt c", p=P)      # [128, 4, 256]
    wvup_r = wvup.rearrange("(kt p) c -> p kt c", p=P)      # [128, 4, 512]
    wo_r = wo.rearrange("(kt p) d -> p kt d", p=P)          # [128, 4, 2048]
    mask_r = maskd.rearrange("p (r i) -> p r i", r=4)       # [128, 4, 512]
    o_r = o_part.rearrange("(st p) d -> p st d", p=P)       # [128, 16, 2048]

    with tile.TileContext(nc) as tc:
        with (
            tc.tile_pool(name="persist", bufs=1) as pp,
            tc.tile_pool(name="acts", bufs=1) as ap_,
            tc.tile_pool(name="wstream", bufs=3) as wp,
            tc.tile_pool(name="wbig", bufs=1) as wb,
            tc.tile_pool(name="tabs", bufs=1) as tp,
            tc.tile_pool(name="rope", bufs=2) as rp,
            tc.tile_pool(name="attn", bufs=2) as atp,
            tc.tile_pool(name="outp", bufs=2) as op_,
            tc.tile_pool(name="aoutp", bufs=2) as aop,
            tc.tile_pool(name="psA", bufs=2, space="PSUM") as psA,
            tc.tile_pool(name="psS", bufs=2, space="PSUM") as psS,
            tc.tile_pool(name="psD", bufs=2, space="PSUM") as psD,
            tc.tile_pool(name="psO", bufs=2, space="PSUM") as psO,
        ):
            # ---------------- persistent tiles ----------------
            kT = pp.tile([P, HLOC, S], BF16, tag="kT")            # per-head K^T
            vnat = pp.tile([P, S // P, HLOC * HD], BF16, tag="vnat")  # V natural
            masks = pp.tile([P, 4, CHUNK], BF16, tag="masks")
            ones = pp.tile([P, P], BF16, tag="ones")
            wo_t = pp.tile([P, 4, D], BF16, tag="wo")             # resident Wo

            nc.vector.memset(ones[:], 1.0)

            def o_proj(ic, aout):
                for st in range(CHUNK // P):
                    for dc in range(D // CHUNK):
                        ps = psA.tile([P, CHUNK], F32, tag="psA")
                        for kt in range(HLOC):
                            nc.tensor.matmul(
                                ps[:], aout[:, kt, P * st:P * (st + 1)],
                                wo_t[:, kt, CHUNK * dc:CHUNK * (dc + 1)],
                                start=(kt == 0), stop=(kt == HLOC - 1))
                        osb = op_.tile([P, CHUNK], F32, tag="osb")
                        nc.scalar.copy(osb[:], ps[:])
                        nc.scalar.dma_start(
                            o_r[:, ic * (CHUNK // P) + st,
                                CHUNK * dc:CHUNK * (dc + 1)], osb[:])

            def rope_store(ps_pe, b, dst_pe, cos_c, sin_c):
                """ps_pe: [64, CHUNK] psum AP at partition base b in {0, 64}
                (pre-rope pe rows of one head). 4 DVE ops, windows chosen so
                the sign-baked sin/cos tables align with tmp/scr rows; only
                psum reads / the final store cross partition quadrants.
                dst_pe = bf16 tile rows [64:128]."""
                tmp = rp.tile([P, CHUNK], F32, tag="ropetmp")
                scr = rp.tile([P, CHUNK], F32, tag="ropescr")
                nc.vector.tensor_tensor(tmp[b:b + 32, :], ps_pe[32:64, :],
                                        sin_c[b:b + 32, :], mybir.AluOpType.mult)
                nc.vector.tensor_tensor(tmp[b + 32:b + 64, :], ps_pe[0:32, :],
                                        sin_c[b + 32:b + 64, :],
                                        mybir.AluOpType.mult)
                nc.vector.tensor_tensor(scr[b:b + 64, :], ps_pe[:],
                                        cos_c[b:b + 64, :], mybir.AluOpType.mult)
                nc.vector.tensor_tensor(dst_pe, scr[b:b + 64, :],
                                        tmp[b:b + 64, :], mybir.AluOpType.add)

            # ---------------- chunk loop ----------------
            for ic in range(NCHUNK):
                sl = slice(ic * CHUNK, (ic + 1) * CHUNK)

                xc = ap_.tile([P, D // P, CHUNK], BF16, tag="xc")
                for dt_ in range(D // P):
                    nc.sync.dma_start(xc[:, dt_, :], xT_r[:, dt_, sl])

                cos_c = tp.tile([P, CHUNK], F32, tag="cos")
                sin_c = tp.tile([P, CHUNK], F32, tag="sin")
                nc.scalar.dma_start(cos_c[:], cosr[:, sl])
                nc.scalar.dma_start(sin_c[:], sinr[:, sl])
                if ic == 0:
                    nc.scalar.dma_start(masks[:], mask_r[:])

                # ---- q_latT [1024, CHUNK] (f32r) ----
                qlat = ap_.tile([P, QR // P, CHUNK], F32R, tag="qlat")
                for cp in range(QR // P // 2):          # c-tile pairs
                    ws = wp.tile([P, D // P, 2 * P], BF16, tag="wstrip")
                    nc.sync.dma_start(
                        ws[:], wqd_r[:, :, 2 * P * cp:2 * P * (cp + 1)])
                    for ci in range(2):
                        c = 2 * cp + ci
                        ps = psA.tile([P, CHUNK], F32, tag="psA")
                        for dt_ in range(D // P):
                            nc.tensor.matmul(
                                ps[:], ws[:, dt_, P * ci:P * (ci + 1)],
                                xc[:, dt_, :],
                                start=(dt_ == 0), stop=(dt_ == D // P - 1))
                        nc.scalar.copy(qlat[:, c, :], ps[:])

                # ---- kv_latT [512, CHUNK] (f32r) ----
                kvlat = ap_.tile([P, KVR // P, CHUNK], F32R, tag="kvlat")
                for cp in range(KVR // P // 2):
                    ws = wp.tile([P, D // P, 2 * P], BF16, tag="wstrip")
                    nc.sync.dma_start(
                        ws[:], wkvd_r[:, :, 2 * P * cp:2 * P * (cp + 1)])
                    for ci in range(2):
                        c = 2 * cp + ci
                        ps = psA.tile([P, CHUNK], F32, tag="psA")
                        for dt_ in range(D // P):
                            nc.tensor.matmul(
                                ps[:], ws[:, dt_, P * ci:P * (ci + 1)],
                                xc[:, dt_, :],
                                start=(dt_ == 0), stop=(dt_ == D // P - 1))
                        nc.scalar.copy(kvlat[:, c, :], ps[:])

                # ---- k_pe: 2 c-tiles of 128 = (heads 2a, 2a+1) rope dims ----
                ws_kr = wp.tile([P, D // P, 2 * P], BF16, tag="wstrip")
                nc.sync.dma_start(ws_kr[:], wkr_r[:])
                for a in range(2):
                    ps = psA.tile([P, CHUNK], F32, tag="psA")
                    for dt_ in range(D // P):
                        nc.tensor.matmul(
                            ps[:], ws_kr[:, dt_, P * a:P * (a + 1)],
                            xc[:, dt_, :],
                            start=(dt_ == 0), stop=(dt_ == D // P - 1))
                    rope_store(ps[0:64, :], 0, kT[64:128, 2 * a, sl], cos_c, sin_c)
                    rope_store(ps[64:128, :], 64, kT[64:128, 2 * a + 1, sl],
                               cos_c, sin_c)

                # ---- k_nope: 2 c-tiles = (heads 2a, 2a+1) nope dims ----
                ws_kn = wp.tile([P, KVR // P, 2 * P], F32R, tag="wstrip")
                nc.sync.dma_start(ws_kn[:], wkup_r[:].bitcast(F32R))
                for a in range(2):
                    ps = psA.tile([P, CHUNK], F32, tag="psA")
                    for kt in range(KVR // P):
                        nc.tensor.matmul(
                            ps[:], ws_kn[:, kt, P * a:P * (a + 1)],
                            kvlat[:, kt, :],
                            start=(kt == 0), stop=(kt == KVR // P - 1))
                    nc.vector.tensor_copy(kT[0:64, 2 * a, sl], ps[0:64, :])
                    nc.vector.tensor_copy(kT[0:64, 2 * a + 1, sl], ps[64:128, :])

                # ---- q heads: c-tile h = head h [nope64 | pe64] ----
                qTi = ap_.tile([P, HLOC, CHUNK], BF16, tag="qTi")
                for hp in range(HLOC // 2):
                    ws = wp.tile([P, QR // P, 2 * P], F32R, tag="wstrip")
                    nc.sync.dma_start(
                        ws[:],
                        wqcat_r[:, :, 2 * P * hp:2 * P * (hp + 1)].bitcast(F32R))
                    for ci in range(2):
                        h = 2 * hp + ci
                        ps = psA.tile([P, CHUNK], F32, tag="psA")
                        for qt in range(QR // P):
                            nc.tensor.matmul(
                                ps[:], ws[:, qt, P * ci:P * (ci + 1)],
                                qlat[:, qt, :],
                                start=(qt == 0), stop=(qt == QR // P - 1))
                        nc.vector.tensor_copy(qTi[0:64, h, :], ps[0:64, :])
                        rope_store(ps[64:128, :], 64, qTi[64:128, h, :], cos_c, sin_c)

                if ic == 0:
                    # resident Wo load, deferred so it doesn't crowd the
                    # critical first-chunk x/weight DMAs
                    for kt in range(4):
                        nc.sync.dma_start(wo_t[:, kt, :], wo_r[:, kt, :])

                # ---- v natural [CHUNK, 512] ----
                ws_v = wb.tile([P, KVR // P, HLOC * HD], F32R, tag="wvup")
                nc.sync.dma_start(ws_v[:], wvup_r[:].bitcast(F32R))
                for st in range(CHUNK // P):
                    ps = psA.tile([P, HLOC * HD], F32, tag="psA")
                    for kt in range(KVR // P):
                        nc.tensor.matmul(
                            ps[:], kvlat[:, kt, P * st:P * (st + 1)],
                            ws_v[:, kt, :],
                            start=(kt == 0), stop=(kt == KVR // P - 1))
                    nc.vector.tensor_copy(vnat[:, ic * (CHUNK // P) + st, :], ps[:])

                # ---- o-projection of the PREVIOUS chunk: PE work to cover
                # the DVE rope/normalize backlog of this chunk's projections
                if ic > 0:
                    o_proj(ic - 1, prev_aout)

                # ---- attention for this query chunk ----
                aout = aop.tile([P, HLOC, CHUNK], BF16, tag="aout")
                jt_max = (ic + 1) * (CHUNK // P)
                for h in range(HLOC):
                    psd = psD.tile([P, CHUNK], F32, tag="psD")
                    pso = psO.tile([P, CHUNK], F32, tag="psO")
                    for jt in range(jt_max):
                        pss = psS.tile([P, CHUNK], F32, tag="psS")
                        nc.tensor.matmul(
                            pss[:], kT[:, h, P * jt:P * (jt + 1)], qTi[:, h, :],
                            start=True, stop=True)
                        at = atp.tile([P, CHUNK], BF16, tag="attnT")
                        nc.scalar.activation(
                            at[:], pss[:], mybir.ActivationFunctionType.Exp,
                            scale=SCALE)
                        r = jt - ic * (CHUNK // P)
                        if r >= 0:  # diagonal tile: causal mask
                            nc.vector.tensor_tensor(
                                at[:], at[:], masks[:, r, :],
                                mybir.AluOpType.mult)
                        nc.tensor.matmul(psd[:], ones[:], at[:],
                                         start=(jt == 0), stop=(jt == jt_max - 1))
                        nc.tensor.matmul(
                            pso[:], vnat[:, jt, HD * h:HD * (h + 1)], at[:],
                            start=(jt == 0), stop=(jt == jt_max - 1))
                    rec = atp.tile([P, CHUNK], F32, tag="recip")
                    nc.vector.reciprocal_approx_fast(rec[:], psd[:])
                    nc.vector.tensor_tensor(aout[:, h, :], pso[:], rec[:],
                                            mybir.AluOpType.mult)
                prev_aout = aout

            o_proj(NCHUNK - 1, prev_aout)
    nc.compile()
    return nc


_NC = None


def _get_nc():
    global _NC
    if _NC is None:
        _NC = _build()
    return _NC


def _host_prep(x, Wq_down, Wq_up, Wq_rope, Wkv_down, Wk_up, Wk_rope, Wv_up, Wo):
    """Build the 8 per-core input maps (all host-side layout prep)."""
    # rope tables, replicated to 128 partitions with NeoX sign baked into sin
    half = RD // 2
    inv_freq = 1.0 / (BASE ** (np.arange(half, dtype=np.float64) / half))
    ang = np.arange(S, dtype=np.float64)[None, :] * inv_freq[:, None]  # [32, S]
    cos32 = np.cos(ang)
    sin32 = np.sin(ang)
    cosr = np.tile(cos32, (4, 1)).astype(np.float32)                   # [128,S]
    sinr = np.concatenate([-sin32, sin32, -sin32, sin32], 0).astype(np.float32)

    # causal masks for diagonal (j-tile, i-chunk) pairs: r = 0..3
    pidx = np.arange(P)[:, None]
    iidx = np.arange(CHUNK)[None, :]
    masks = np.stack([(pidx + P * r <= iidx) for r in range(4)], axis=1)
    maskd = masks.astype(_BF16).reshape(P, 4 * CHUNK)

    xT = [np.ascontiguousarray(x[b].T).astype(_BF16) for b in range(B)]
    wqd = Wq_down.astype(_BF16)
    wkvd = Wkv_down.astype(_BF16)

    in_maps = []
    for c in range(8):
        b, g = divmod(c, 4)
        heads = range(HLOC * g, HLOC * (g + 1))
        wqcat = np.empty((QR, HLOC * HD), np.float32)
        for i, h in enumerate(heads):
            wqcat[:, i * HD:i * HD + ND] = Wq_up[:, h * ND:(h + 1) * ND]
            wqcat[:, i * HD + ND:(i + 1) * HD] = Wq_rope[:, h * RD:(h + 1) * RD]
        in_maps.append({
            "xT": xT[b],
            "wqd": wqd,
            "wkvd": wkvd,
            "wkr": np.ascontiguousarray(
                Wk_rope[:, g * HLOC * RD:(g + 1) * HLOC * RD]).astype(_BF16),
            "wqcat": wqcat,
            "wkup": np.ascontiguousarray(
                Wk_up[:, g * HLOC * ND:(g + 1) * HLOC * ND], dtype=np.float32),
            "wvup": np.ascontiguousarray(
                Wv_up[:, g * HLOC * HD:(g + 1) * HLOC * HD], dtype=np.float32),
            "wo": np.ascontiguousarray(
                Wo[g * HLOC * HD:(g + 1) * HLOC * HD, :]).astype(_BF16),
            "cosr": cosr,
            "sinr": sinr,
            "maskd": maskd,
        })
    return in_maps


def kernel(x, Wq_down, Wq_up, Wq_rope, Wkv_down, Wk_up, Wk_rope, Wv_up, Wo,
           _trace=False, _trace_kwargs=None):
    x = np.asarray(x, dtype=np.float32)
    args = [np.asarray(a, dtype=np.float32) for a in
            (Wq_down, Wq_up, Wq_rope, Wkv_down, Wk_up, Wk_rope, Wv_up, Wo)]
    in_maps = _host_prep(x, *args)
    nc = _get_nc()
    res = run_bass_kernel_spmd(nc, in_maps, core_ids=list(range(8)),
                               trace=_trace, **(_trace_kwargs or {}))
    kernel._last_results = res
    out = np.zeros((B, S, D), np.float32)
    for c in range(8):
        out[c // 4] += res.results[c]["o_part"]
    return out



# revision 3
# speedup vs baseline: 1.2296x; 1.2296x over previous
"""MLA attention (DeepSeek-style) Trainium2 Bass kernel, 8-core SPMD.

Sharding: core c handles batch b = c//4 and head-group g = c%4 (4 of 16 heads).
Down-projections are replicated per batch; up-projections / attention / o-proj
are head-parallel. Host sums the 4 partial o-projections per batch.

v2 schedule (vs baseline): all activations double-buffered so chunk ic+1's
down-projections interleave into chunk ic's attention stalls; rope is
decoupled from PSUM via ScalarE drains to SBUF + bf16 DVE math; kT/vnat are
per-chunk tiles (no false cross-chunk deps); causal diagonal is trimmed at
128 granularity; psS has 3 PSUM banks so the scores->exp->attnout pipeline
runs at exp rate; small weights resident in SBUF; bf16 latents and output
partials.
"""

import numpy as np
import ml_dtypes

import concourse.bacc as bacc
import concourse.mybir as mybir
import concourse.tile as tile
from concourse.bass_utils import run_bass_kernel_spmd

F32 = mybir.dt.float32
BF16 = mybir.dt.bfloat16

B, S, D = 2, 2048, 2048
H, HD = 16, 128
RD, ND = 64, 64
KVR, QR = 512, 1024
BASE = 10000.0
HLOC = 4                 # heads per core
CHUNK = 512
NCHUNK = S // CHUNK      # 4
P = 128
SCALE = HD ** -0.5

_BF16 = ml_dtypes.bfloat16


def _build():
    nc = bacc.Bacc("TRN2", target_bir_lowering=False, debug=False)

    xT = nc.dram_tensor("xT", [D, S], BF16, kind="ExternalInput").ap()
    wqd = nc.dram_tensor("wqd", [D, QR], BF16, kind="ExternalInput").ap()
    wkvd = nc.dram_tensor("wkvd", [D, KVR], BF16, kind="ExternalInput").ap()
    wkr = nc.dram_tensor("wkr", [D, HLOC * RD], BF16, kind="ExternalInput").ap()
    wqcat = nc.dram_tensor("wqcat", [QR, HLOC * HD], BF16, kind="ExternalInput").ap()
    wkup = nc.dram_tensor("wkup", [KVR, HLOC * ND], BF16, kind="ExternalInput").ap()
    wvup = nc.dram_tensor("wvup", [KVR, HLOC * HD], BF16, kind="ExternalInput").ap()
    wo = nc.dram_tensor("wo", [HLOC * HD, D], BF16, kind="ExternalInput").ap()
    cosr = nc.dram_tensor("cosr", [P, S], BF16, kind="ExternalInput").ap()
    sinr = nc.dram_tensor("sinr", [P, S], BF16, kind="ExternalInput").ap()
    maskd = nc.dram_tensor("maskd", [P, P], BF16, kind="ExternalInput").ap()
    o_part = nc.dram_tensor("o_part", [S, D], BF16, kind="ExternalOutput").ap()

    xT_r = xT.rearrange("(dt p) s -> p dt s", p=P)          # [128, 16, S]
    wqd_r = wqd.rearrange("(dt p) q -> p dt q", p=P)        # [128, 16, 1024]
    wkvd_r = wkvd.rearrange("(dt p) q -> p dt q", p=P)      # [128, 16, 512]
    wkr_r = wkr.rearrange("(dt p) q -> p dt q", p=P)        # [128, 16, 256]
    wqcat_r = wqcat.rearrange("(qt p) c -> p qt c", p=P)    # [128, 8, 512]
    wkup_r = wkup.rearrange("(kt p) c -> p kt c", p=P)      # [128, 4, 256]
    wvup_r = wvup.rearrange("(kt p) c -> p kt c", p=P)      # [128, 4, 512]
    wo_r = wo.rearrange("(kt p) d -> p kt d", p=P)          # [128, 4, 2048]
    o_r = o_part.rearrange("(st p) d -> p st d", p=P)       # [128, 16, 2048]

    with tile.TileContext(nc) as tc:
        with (
            tc.tile_pool(name="persist", bufs=1) as pp,
            tc.tile_pool(name="acts", bufs=2) as ap_,
            tc.tile_pool(name="wstream", bufs=3) as wp,
            tc.tile_pool(name="rope", bufs=2) as rp,
            tc.tile_pool(name="attn", bufs=3) as atp,
            tc.tile_pool(name="recp", bufs=2) as rcp,
            tc.tile_pool(name="outp", bufs=2) as op_,
            tc.tile_pool(name="aoutp", bufs=2) as aop,
            tc.tile_pool(name="psA", bufs=2, space="PSUM") as psA,
            tc.tile_pool(name="psS", bufs=3, space="PSUM") as psS,
            tc.tile_pool(name="psD", bufs=1, space="PSUM") as psD,
            tc.tile_pool(name="psO", bufs=2, space="PSUM") as psO,
        ):
            # ---------------- persistent tiles ----------------
            kT = [pp.tile([P, HLOC, CHUNK], BF16, name=f"kT{j}", tag=f"kT{j}")
                  for j in range(NCHUNK)]                     # per-chunk K^T
            vnat = [pp.tile([P, CHUNK // P, HLOC * HD], BF16, name=f"vn{j}", tag=f"vn{j}")
                    for j in range(NCHUNK)]                   # per-chunk V nat
            mask = pp.tile([P, P], BF16, tag="mask")
            ones = pp.tile([P, P], BF16, tag="ones")
            wo_t = pp.tile([P, HLOC, D], BF16, tag="wo")
            wqc_t = pp.tile([P, QR // P, HLOC * HD], BF16, tag="wqc")
            wku_t = pp.tile([P, KVR // P, HLOC * ND], BF16, tag="wku")
            wvu_t = pp.tile([P, KVR // P, HLOC * HD], BF16, tag="wvu")
            wkr_t = pp.tile([P, D // P, HLOC * RD], BF16, tag="wkr")
            cos_t = pp.tile([P, S], BF16, tag="cos")
            sin_t = pp.tile([P, S], BF16, tag="sin")

            nc.vector.memset(ones[:], 1.0)
            # resident loads (gpsimd queue: off the critical sync path)
            nc.gpsimd.dma_start(mask[:], maskd[:])
            nc.gpsimd.dma_start(cos_t[:], cosr[:])
            nc.gpsimd.dma_start(sin_t[:], sinr[:])
            nc.gpsimd.dma_start(wqc_t[:], wqcat_r[:])
            nc.gpsimd.dma_start(wku_t[:], wkup_r[:])
            nc.gpsimd.dma_start(wvu_t[:], wvup_r[:])
            nc.gpsimd.dma_start(wkr_t[:], wkr_r[:])
            for kt_ in range(HLOC):
                nc.gpsimd.dma_start(wo_t[:, kt_, :], wo_r[:, kt_, :])

            def o_proj(ic, aout):
                """Project previous chunk's attention output; PE filler work
                emitted just before the attention loop."""
                for st in range(CHUNK // P):
                    osb = op_.tile([P, D], BF16, tag="osb")
                    for dc in range(D // CHUNK):
                        ps = psA.tile([P, CHUNK], F32, tag="psA")
                        for kt_ in range(HLOC):
                            nc.tensor.matmul(
                                ps[:], aout[:, kt_, P * st:P * (st + 1)],
                                wo_t[:, kt_, CHUNK * dc:CHUNK * (dc + 1)],
                                start=(kt_ == 0), stop=(kt_ == HLOC - 1))
                        nc.vector.tensor_copy(
                            osb[:, CHUNK * dc:CHUNK * (dc + 1)], ps[:])
                    nc.gpsimd.dma_start(
                        o_r[:, ic * (CHUNK // P) + st, :], osb[:])

            # ---------------- chunk loop ----------------
            for ic in range(NCHUNK):
                sl = slice(ic * CHUNK, (ic + 1) * CHUNK)

                xc = ap_.tile([P, D // P, CHUNK], BF16, tag="xc")
                nc.sync.dma_start(xc[:], xT_r[:, :, sl])

                # ---- q_latT [1024, CHUNK] (bf16) ----
                qlat = ap_.tile([P, QR // P, CHUNK], BF16, tag="qlat")
                for cp in range(QR // P // 2):          # c-tile pairs
                    ws = wp.tile([P, D // P, 2 * P], BF16, tag="wstrip")
                    nc.sync.dma_start(
                        ws[:], wqd_r[:, :, 2 * P * cp:2 * P * (cp + 1)])
                    for ci in range(2):
                        c = 2 * cp + ci
                        ps = psA.tile([P, CHUNK], F32, tag="psA")
                        for dt_ in range(D // P):
                            nc.tensor.matmul(
                                ps[:], ws[:, dt_, P * ci:P * (ci + 1)],
                                xc[:, dt_, :],
                                start=(dt_ == 0), stop=(dt_ == D // P - 1))
                        nc.scalar.copy(qlat[:, c, :], ps[:])

                # ---- kv_latT [512, CHUNK] (bf16) ----
                kvlat = ap_.tile([P, KVR // P, CHUNK], BF16, tag="kvlat")
                for cp in range(KVR // P // 2):
                    ws = wp.tile([P, D // P, 2 * P], BF16, tag="wstrip")
                    nc.sync.dma_start(
                        ws[:], wkvd_r[:, :, 2 * P * cp:2 * P * (cp + 1)])
                    for ci in range(2):
                        c = 2 * cp + ci
                        ps = psA.tile([P, CHUNK], F32, tag="psA")
                        for dt_ in range(D // P):
                            nc.tensor.matmul(
                                ps[:], ws[:, dt_, P * ci:P * (ci + 1)],
                                xc[:, dt_, :],
                                start=(dt_ == 0), stop=(dt_ == D // P - 1))
                        nc.scalar.copy(kvlat[:, c, :], ps[:])

                cos_c = cos_t[:, sl]
                sin_c = sin_t[:, sl]

                # ---- k_pe: head pair a -> heads (2a, 2a+1) rope dims ----
                # drain psum via ScalarE to SBUF bf16, rope on DVE from SBUF
                for a in range(2):
                    ps = psA.tile([P, CHUNK], F32, tag="psA")
                    for dt_ in range(D // P):
                        nc.tensor.matmul(
                            ps[:], wkr_t[:, dt_, P * a:P * (a + 1)],
                            xc[:, dt_, :],
                            start=(dt_ == 0), stop=(dt_ == D // P - 1))
                    raw = rp.tile([P, CHUNK], BF16, tag="kraw")
                    sh = rp.tile([P, CHUNK], BF16, tag="ksh")
                    scr = rp.tile([P, CHUNK], BF16, tag="kscr")
                    nc.scalar.copy(raw[:], ps[:])
                    # NeoX rotation: shifted halves within each 64-row block
                    for b in (0, 64):
                        nc.vector.tensor_copy(sh[b:b + 32, :],
                                              raw[b + 32:b + 64, :])
                        nc.vector.tensor_copy(sh[b + 32:b + 64, :],
                                              raw[b:b + 32, :])
                    nc.vector.tensor_tensor(sh[:], sh[:], sin_c,
                                            mybir.AluOpType.mult)
                    nc.vector.tensor_tensor(scr[:], raw[:], cos_c,
                                            mybir.AluOpType.mult)
                    nc.vector.tensor_tensor(kT[ic][64:128, 2 * a, :],
                                            scr[0:64, :], sh[0:64, :],
                                            mybir.AluOpType.add)
                    nc.vector.tensor_tensor(kT[ic][64:128, 2 * a + 1, :],
                                            scr[64:128, :], sh[64:128, :],
                                            mybir.AluOpType.add)

                # ---- q heads: c-tile h = head h [nope64 | pe64] ----
                qTi = ap_.tile([P, HLOC, CHUNK], BF16, tag="qTi")
                for h in range(HLOC):
                    ps = psA.tile([P, CHUNK], F32, tag="psA")
                    for qt in range(QR // P):
                        nc.tensor.matmul(
                            ps[:], wqc_t[:, qt, P * h:P * (h + 1)],
                            qlat[:, qt, :],
                            start=(qt == 0), stop=(qt == QR // P - 1))
                    nc.scalar.copy(qTi[0:64, h, :], ps[0:64, :])
                    raw = rp.tile([P, CHUNK], BF16, tag="qraw")
                    sh = rp.tile([P, CHUNK], BF16, tag="qsh")
                    scr = rp.tile([P, CHUNK], BF16, tag="qscr")
                    nc.scalar.copy(raw[64:128, :], ps[64:128, :])
                    nc.vector.tensor_copy(sh[64:96, :], raw[96:128, :])
                    nc.vector.tensor_copy(sh[96:128, :], raw[64:96, :])
                    nc.vector.tensor_tensor(sh[64:128, :], sh[64:128, :],
                                            sin_c[64:128, :],
                                            mybir.AluOpType.mult)
                    nc.vector.tensor_tensor(scr[64:128, :], raw[64:128, :],
                                            cos_c[64:128, :],
                                            mybir.AluOpType.mult)
                    nc.vector.tensor_tensor(qTi[64:128, h, :],
                                            scr[64:128, :], sh[64:128, :],
                                            mybir.AluOpType.add)

                # ---- k_nope: head pair a -> heads (2a, 2a+1) nope dims ----
                for a in range(2):
                    ps = psA.tile([P, CHUNK], F32, tag="psA")
                    for kt_ in range(KVR // P):
                        nc.tensor.matmul(
                            ps[:], wku_t[:, kt_, P * a:P * (a + 1)],
                            kvlat[:, kt_, :],
                            start=(kt_ == 0), stop=(kt_ == KVR // P - 1))
                    nc.vector.tensor_copy(kT[ic][0:64, 2 * a, :], ps[0:64, :])
                    nc.vector.tensor_copy(kT[ic][0:64, 2 * a + 1, :],
                                          ps[64:128, :])

                # ---- v natural [CHUNK, 512] ----
                for st in range(CHUNK // P):
                    ps = psA.tile([P, HLOC * HD], F32, tag="psA")
                    for kt_ in range(KVR // P):
                        nc.tensor.matmul(
                            ps[:], kvlat[:, kt_, P * st:P * (st + 1)],
                            wvu_t[:, kt_, :],
                            start=(kt_ == 0), stop=(kt_ == KVR // P - 1))
                    nc.vector.tensor_copy(vnat[ic][:, st, :], ps[:])

                # ---- o-projection of the PREVIOUS chunk: PE filler that
                # covers this chunk's rope/DVE latency before attention
                if ic > 0:
                    o_proj(ic - 1, prev_aout)

                # ---- attention for this query chunk (diagonal trimmed) ----
                aout = aop.tile([P, HLOC, CHUNK], BF16, tag="aout")
                for h in range(HLOC):
                    psd = psD.tile([P, CHUNK], F32, tag="psD")
                    pso = psO.tile([P, CHUNK], F32, tag="psO")
                    nj = 4 * ic + 4            # total j-tiles incl. diagonal
                    for jt in range(nj):
                        jc, r = divmod(jt, 4)
                        diag = jc == ic
                        off = P * r if diag else 0      # first query col
                        n = CHUNK - off
                        first, last = jt == 0, jt == nj - 1
                        pss = psS.tile([P, CHUNK], F32, tag="psS")
                        nc.tensor.matmul(
                            pss[:, off:], kT[jc][:, h, P * r:P * (r + 1)],
                            qTi[:, h, off:], start=True, stop=True)
                        at = atp.tile([P, CHUNK], BF16, tag="attnT")
                        nc.scalar.activation(
                            at[:, off:], pss[:, off:],
                            mybir.ActivationFunctionType.Exp, scale=SCALE)
                        if diag:
                            nc.vector.tensor_tensor(
                                at[:, off:off + P], at[:, off:off + P],
                                mask[:], mybir.AluOpType.mult)
                        nc.tensor.matmul(
                            pso[:, off:], vnat[jc][:, r, HD * h:HD * (h + 1)],
                            at[:, off:], start=first, stop=last)
                        nc.tensor.matmul(psd[:, off:], ones[:], at[:, off:],
                                         start=first, stop=last)
                    rec = rcp.tile([P, CHUNK], F32, tag="recip")
                    nc.vector.reciprocal_approx_fast(rec[:], psd[:])
                    nc.vector.tensor_tensor(aout[:, h, :], pso[:], rec[:],
                                            mybir.AluOpType.mult)
                prev_aout = aout

            o_proj(NCHUNK - 1, prev_aout)
    nc.compile()
    return nc


_NC = None


def _get_nc():
    global _NC
    if _NC is None:
        _NC = _build()
    return _NC


def _host_prep(x, Wq_down, Wq_up, Wq_rope, Wkv_down, Wk_up, Wk_rope, Wv_up, Wo):
    """Build the 8 per-core input maps (all host-side layout prep)."""
    # rope tables, replicated to 128 partitions with NeoX sign baked into sin
    half = RD // 2
    inv_freq = 1.0 / (BASE ** (np.arange(half, dtype=np.float64) / half))
    ang = np.arange(S, dtype=np.float64)[None, :] * inv_freq[:, None]  # [32, S]
    cos32 = np.cos(ang)
    sin32 = np.sin(ang)
    cosr = np.tile(cos32, (4, 1)).astype(_BF16)                        # [128,S]
    sinr = np.concatenate([-sin32, sin32, -sin32, sin32], 0).astype(_BF16)

    # causal mask for the 128x128 diagonal block: key p visible to query c
    pidx = np.arange(P)[:, None]
    cidx = np.arange(P)[None, :]
    maskd = (pidx <= cidx).astype(_BF16)

    xT = [np.ascontiguousarray(x[b].T).astype(_BF16) for b in range(B)]
    wqd = Wq_down.astype(_BF16)
    wkvd = Wkv_down.astype(_BF16)

    in_maps = []
    for c in range(8):
        b, g = divmod(c, 4)
        heads = range(HLOC * g, HLOC * (g + 1))
        wqcat = np.empty((QR, HLOC * HD), np.float32)
        for i, h in enumerate(heads):
            wqcat[:, i * HD:i * HD + ND] = Wq_up[:, h * ND:(h + 1) * ND]
            wqcat[:, i * HD + ND:(i + 1) * HD] = Wq_rope[:, h * RD:(h + 1) * RD]
        in_maps.append({
            "xT": xT[b],
            "wqd": wqd,
            "wkvd": wkvd,
            "wkr": np.ascontiguousarray(
                Wk_rope[:, g * HLOC * RD:(g + 1) * HLOC * RD]).astype(_BF16),
            "wqcat": wqcat.astype(_BF16),
            "wkup": np.ascontiguousarray(
                Wk_up[:, g * HLOC * ND:(g + 1) * HLOC * ND]).astype(_BF16),
            "wvup": np.ascontiguousarray(
                Wv_up[:, g * HLOC * HD:(g + 1) * HLOC * HD]).astype(_BF16),
            "wo": np.ascontiguousarray(
                Wo[g * HLOC * HD:(g + 1) * HLOC * HD, :]).astype(_BF16),
            "cosr": cosr,
            "sinr": sinr,
            "maskd": maskd,
        })
    return in_maps


def kernel(x, Wq_down, Wq_up, Wq_rope, Wkv_down, Wk_up, Wk_rope, Wv_up, Wo,
           _trace=False, _trace_kwargs=None):
    x = np.asarray(x, dtype=np.float32)
    args = [np.asarray(a, dtype=np.float32) for a in
            (Wq_down, Wq_up, Wq_rope, Wkv_down, Wk_up, Wk_rope, Wv_up, Wo)]
    in_maps = _host_prep(x, *args)
    nc = _get_nc()
    res = run_bass_kernel_spmd(nc, in_maps, core_ids=list(range(8)),
                               trace=_trace, **(_trace_kwargs or {}))
    kernel._last_results = res
    out = np.zeros((B, S, D), np.float32)
    for c in range(8):
        out[c // 4] += res.results[c]["o_part"].astype(np.float32)
    return out


# revision 7
# speedup vs baseline: 1.2816x; 1.0423x over previous
"""MLA attention (DeepSeek-style) Trainium2 Bass kernel, 8-core SPMD.

Sharding: core c handles batch b = c//4 and head-group g = c%4 (4 of 16 heads).
Down-projections are replicated per batch; up-projections / attention / o-proj
are head-parallel. Host sums the 4 partial o-projections per batch.

v2 schedule (vs baseline): all activations double-buffered so chunk ic+1's
down-projections interleave into chunk ic's attention stalls; rope is
decoupled from PSUM via ScalarE drains to SBUF + bf16 DVE math; kT/vnat are
per-chunk tiles (no false cross-chunk deps); causal diagonal is trimmed at
128 granularity; psS has 3 PSUM banks so the scores->exp->attnout pipeline
runs at exp rate; small weights resident in SBUF; bf16 latents and output
partials.
"""

import numpy as np
import ml_dtypes

import concourse.bacc as bacc
import concourse.mybir as mybir
import concourse.tile as tile
from concourse.bass_utils import run_bass_kernel_spmd

F32 = mybir.dt.float32
BF16 = mybir.dt.bfloat16

B, S, D = 2, 2048, 2048
H, HD = 16, 128
RD, ND = 64, 64
KVR, QR = 512, 1024
BASE = 10000.0
HLOC = 4                 # heads per core
CHUNK = 512
NCHUNK = S // CHUNK      # 4
P = 128
SCALE = HD ** -0.5

_BF16 = ml_dtypes.bfloat16


def _build():
    nc = bacc.Bacc("TRN2", target_bir_lowering=False, debug=False)

    xT = nc.dram_tensor("xT", [D, S], BF16, kind="ExternalInput").ap()
    wqd = nc.dram_tensor("wqd", [D, QR], BF16, kind="ExternalInput").ap()
    wkvd = nc.dram_tensor("wkvd", [D, KVR], BF16, kind="ExternalInput").ap()
    wkr = nc.dram_tensor("wkr", [D, HLOC * RD], BF16, kind="ExternalInput").ap()
    wqcat = nc.dram_tensor("wqcat", [QR, HLOC * HD], BF16, kind="ExternalInput").ap()
    wkup = nc.dram_tensor("wkup", [KVR, HLOC * ND], BF16, kind="ExternalInput").ap()
    wvup = nc.dram_tensor("wvup", [KVR, HLOC * HD], BF16, kind="ExternalInput").ap()
    wo = nc.dram_tensor("wo", [HLOC * HD, D], BF16, kind="ExternalInput").ap()
    cosr = nc.dram_tensor("cosr", [P, S], BF16, kind="ExternalInput").ap()
    sinr = nc.dram_tensor("sinr", [P, S], BF16, kind="ExternalInput").ap()
    maskd = nc.dram_tensor("maskd", [P, P], BF16, kind="ExternalInput").ap()
    o_part = nc.dram_tensor("o_part", [S, D], BF16, kind="ExternalOutput").ap()

    xT_r = xT.rearrange("(dt p) s -> p dt s", p=P)          # [128, 16, S]
    wqd_r = wqd.rearrange("(dt p) q -> p dt q", p=P)        # [128, 16, 1024]
    wkvd_r = wkvd.rearrange("(dt p) q -> p dt q", p=P)      # [128, 16, 512]
    wkr_r = wkr.rearrange("(dt p) q -> p dt q", p=P)        # [128, 16, 256]
    wqcat_r = wqcat.rearrange("(qt p) c -> p qt c", p=P)    # [128, 8, 512]
    wkup_r = wkup.rearrange("(kt p) c -> p kt c", p=P)      # [128, 4, 256]
    wvup_r = wvup.rearrange("(kt p) c -> p kt c", p=P)      # [128, 4, 512]
    wo_r = wo.rearrange("(kt p) d -> p kt d", p=P)          # [128, 4, 2048]
    o_r = o_part.rearrange("(st p) d -> p st d", p=P)       # [128, 16, 2048]

    with tile.TileContext(nc) as tc:
        with (
            tc.tile_pool(name="persist", bufs=1) as pp,
            tc.tile_pool(name="acts", bufs=2) as ap_,
            tc.tile_pool(name="wstream", bufs=3) as wp,
            tc.tile_pool(name="rope", bufs=2) as rp,
            tc.tile_pool(name="attn", bufs=3) as atp,
            tc.tile_pool(name="recp", bufs=2) as rcp,
            tc.tile_pool(name="outp", bufs=2) as op_,
            tc.tile_pool(name="aoutp", bufs=2) as aop,
            tc.tile_pool(name="psA", bufs=2, space="PSUM") as psA,
            tc.tile_pool(name="psS", bufs=3, space="PSUM") as psS,
            tc.tile_pool(name="psD", bufs=1, space="PSUM") as psD,
            tc.tile_pool(name="psO", bufs=2, space="PSUM") as psO,
        ):
            # ---------------- persistent tiles ----------------
            kT = [pp.tile([P, HLOC, CHUNK], BF16, name=f"kT{j}", tag=f"kT{j}")
                  for j in range(NCHUNK)]                     # per-chunk K^T
            vnat = [pp.tile([P, CHUNK // P, HLOC * HD], BF16, name=f"vn{j}", tag=f"vn{j}")
                    for j in range(NCHUNK)]                   # per-chunk V nat
            mask = pp.tile([P, P], BF16, tag="mask")
            ones = pp.tile([P, P], BF16, tag="ones")
            wo_t = pp.tile([P, HLOC, D], BF16, tag="wo")
            wqc_t = pp.tile([P, QR // P, HLOC * HD], BF16, tag="wqc")
            wku_t = pp.tile([P, KVR // P, HLOC * ND], BF16, tag="wku")
            wvu_t = pp.tile([P, KVR // P, HLOC * HD], BF16, tag="wvu")
            wkr_t = pp.tile([P, D // P, HLOC * RD], BF16, tag="wkr")
            cos_t = pp.tile([P, S], BF16, tag="cos")
            sin_t = pp.tile([P, S], BF16, tag="sin")

            nc.vector.memset(ones[:], 1.0)

            def load_residents():
                """Emitted after chunk 0's critical x/weight DMAs so these
                don't compete for HBM bandwidth before the first matmul;
                ordered by first use."""
                nc.sync.dma_start(mask[:], maskd[:])
                nc.sync.dma_start(cos_t[:], cosr[:])
                nc.sync.dma_start(sin_t[:], sinr[:])
                nc.sync.dma_start(wkr_t[:], wkr_r[:])
                nc.sync.dma_start(wqc_t[:], wqcat_r[:])
                nc.sync.dma_start(wku_t[:], wkup_r[:])
                nc.sync.dma_start(wvu_t[:], wvup_r[:])
                for kt_ in range(HLOC):
                    nc.sync.dma_start(wo_t[:, kt_, :], wo_r[:, kt_, :])

            def o_proj(ic, aout):
                """Project previous chunk's attention output; PE filler work
                emitted just before the attention loop."""
                for st in range(CHUNK // P):
                    osb = op_.tile([P, D], BF16, tag="osb")
                    for dc in range(D // CHUNK):
                        ps = psA.tile([P, CHUNK], F32, tag="psA")
                        for kt_ in range(HLOC):
                            nc.tensor.matmul(
                                ps[:], aout[:, kt_, P * st:P * (st + 1)],
                                wo_t[:, kt_, CHUNK * dc:CHUNK * (dc + 1)],
                                start=(kt_ == 0), stop=(kt_ == HLOC - 1))
                        nc.vector.tensor_copy(
                            osb[:, CHUNK * dc:CHUNK * (dc + 1)], ps[:])
                        # per-dc store so the final drain overlaps compute
                        nc.gpsimd.dma_start(
                            o_r[:, ic * (CHUNK // P) + st,
                                CHUNK * dc:CHUNK * (dc + 1)],
                            osb[:, CHUNK * dc:CHUNK * (dc + 1)])

            # ---------------- chunk loop ----------------
            for ic in range(NCHUNK):
                sl = slice(ic * CHUNK, (ic + 1) * CHUNK)

                # first weight strip before the bulky x load so the first
                # matmul's operands arrive earliest
                ws0 = wp.tile([P, D // P, 2 * P], BF16, name="ws0",
                              tag="wstrip")
                nc.sync.dma_start(ws0[:], wqd_r[:, :, 0:2 * P])
                xc = ap_.tile([P, D // P, CHUNK], BF16, tag="xc")
                nc.sync.dma_start(xc[:, 0:8, :], xT_r[:, 0:8, sl])
                nc.sync.dma_start(xc[:, 8:16, :], xT_r[:, 8:16, sl])

                # ---- q_latT [1024, CHUNK] (bf16) ----
                qlat = ap_.tile([P, QR // P, CHUNK], BF16, tag="qlat")
                for cp in range(QR // P // 2):          # c-tile pairs
                    if cp == 0:
                        ws = ws0
                    else:
                        ws = wp.tile([P, D // P, 2 * P], BF16, tag="wstrip")
                        nc.sync.dma_start(
                            ws[:], wqd_r[:, :, 2 * P * cp:2 * P * (cp + 1)])
                    for ci in range(2):
                        c = 2 * cp + ci
                        ps = psA.tile([P, CHUNK], F32, tag="psA")
                        for dt_ in range(D // P):
                            nc.tensor.matmul(
                                ps[:], ws[:, dt_, P * ci:P * (ci + 1)],
                                xc[:, dt_, :],
                                start=(dt_ == 0), stop=(dt_ == D // P - 1))
                        nc.scalar.copy(qlat[:, c, :], ps[:])

                # ---- kv_latT [512, CHUNK] (bf16) ----
                kvlat = ap_.tile([P, KVR // P, CHUNK], BF16, tag="kvlat")
                for cp in range(KVR // P // 2):
                    ws = wp.tile([P, D // P, 2 * P], BF16, tag="wstrip")
                    nc.sync.dma_start(
                        ws[:], wkvd_r[:, :, 2 * P * cp:2 * P * (cp + 1)])
                    for ci in range(2):
                        c = 2 * cp + ci
                        ps = psA.tile([P, CHUNK], F32, tag="psA")
                        for dt_ in range(D // P):
                            nc.tensor.matmul(
                                ps[:], ws[:, dt_, P * ci:P * (ci + 1)],
                                xc[:, dt_, :],
                                start=(dt_ == 0), stop=(dt_ == D // P - 1))
                        nc.scalar.copy(kvlat[:, c, :], ps[:])

                if ic == 0:
                    load_residents()

                cos_c = cos_t[:, sl]
                sin_c = sin_t[:, sl]

                # ---- k_pe: head pair a -> heads (2a, 2a+1) rope dims ----
                # drain psum via ScalarE to SBUF bf16, rope on DVE from SBUF
                for a in range(2):
                    ps = psA.tile([P, CHUNK], F32, tag="psA")
                    for dt_ in range(D // P):
                        nc.tensor.matmul(
                            ps[:], wkr_t[:, dt_, P * a:P * (a + 1)],
                            xc[:, dt_, :],
                            start=(dt_ == 0), stop=(dt_ == D // P - 1))
                    raw = rp.tile([P, CHUNK], BF16, tag="kraw")
                    sh = rp.tile([P, CHUNK], BF16, tag="ksh")
                    scr = rp.tile([P, CHUNK], BF16, tag="kscr")
                    nc.scalar.copy(raw[:], ps[:])
                    # NeoX rotation: shifted halves within each 64-row block
                    for b in (0, 64):
                        nc.vector.tensor_copy(sh[b:b + 32, :],
                                              raw[b + 32:b + 64, :])
                        nc.vector.tensor_copy(sh[b + 32:b + 64, :],
                                              raw[b:b + 32, :])
                    nc.vector.tensor_tensor(sh[:], sh[:], sin_c,
                                            mybir.AluOpType.mult)
                    nc.vector.tensor_tensor(scr[:], raw[:], cos_c,
                                            mybir.AluOpType.mult)
                    nc.vector.tensor_tensor(kT[ic][64:128, 2 * a, :],
                                            scr[0:64, :], sh[0:64, :],
                                            mybir.AluOpType.add)
                    nc.vector.tensor_tensor(kT[ic][64:128, 2 * a + 1, :],
                                            scr[64:128, :], sh[64:128, :],
                                            mybir.AluOpType.add)

                # ---- q heads: c-tile h = head h [nope64 | pe64] ----
                qTi = ap_.tile([P, HLOC, CHUNK], BF16, tag="qTi")
                for h in range(HLOC):
                    ps = psA.tile([P, CHUNK], F32, tag="psA")
                    for qt in range(QR // P):
                        nc.tensor.matmul(
                            ps[:], wqc_t[:, qt, P * h:P * (h + 1)],
                            qlat[:, qt, :],
                            start=(qt == 0), stop=(qt == QR // P - 1))
                    nc.scalar.copy(qTi[0:64, h, :], ps[0:64, :])
                    raw = rp.tile([P, CHUNK], BF16, tag="qraw")
                    sh = rp.tile([P, CHUNK], BF16, tag="qsh")
                    scr = rp.tile([P, CHUNK], BF16, tag="qscr")
                    nc.scalar.copy(raw[64:128, :], ps[64:128, :])
                    nc.vector.tensor_copy(sh[64:96, :], raw[96:128, :])
                    nc.vector.tensor_copy(sh[96:128, :], raw[64:96, :])
                    nc.vector.tensor_tensor(sh[64:128, :], sh[64:128, :],
                                            sin_c[64:128, :],
                                            mybir.AluOpType.mult)
                    nc.vector.tensor_tensor(scr[64:128, :], raw[64:128, :],
                                            cos_c[64:128, :],
                                            mybir.AluOpType.mult)
                    nc.vector.tensor_tensor(qTi[64:128, h, :],
                                            scr[64:128, :], sh[64:128, :],
                                            mybir.AluOpType.add)

                # ---- k_nope: head pair a -> heads (2a, 2a+1) nope dims ----
                for a in range(2):
                    ps = psA.tile([P, CHUNK], F32, tag="psA")
                    for kt_ in range(KVR // P):
                        nc.tensor.matmul(
                            ps[:], wku_t[:, kt_, P * a:P * (a + 1)],
                            kvlat[:, kt_, :],
                            start=(kt_ == 0), stop=(kt_ == KVR // P - 1))
                    nc.vector.tensor_copy(kT[ic][0:64, 2 * a, :], ps[0:64, :])
                    nc.vector.tensor_copy(kT[ic][0:64, 2 * a + 1, :],
                                          ps[64:128, :])

                # ---- v natural [CHUNK, 512] ----
                for st in range(CHUNK // P):
                    ps = psA.tile([P, HLOC * HD], F32, tag="psA")
                    for kt_ in range(KVR // P):
                        nc.tensor.matmul(
                            ps[:], kvlat[:, kt_, P * st:P * (st + 1)],
                            wvu_t[:, kt_, :],
                            start=(kt_ == 0), stop=(kt_ == KVR // P - 1))
                    nc.vector.tensor_copy(vnat[ic][:, st, :], ps[:])

                # ---- o-projection of the PREVIOUS chunk: PE filler that
                # covers this chunk's rope/DVE latency before attention
                if ic > 0:
                    o_proj(ic - 1, prev_aout)

                # ---- attention for this query chunk (diagonal trimmed) ----
                aout = aop.tile([P, HLOC, CHUNK], BF16, tag="aout")
                for h in range(HLOC):
                    psd = psD.tile([P, CHUNK], F32, tag="psD")
                    pso = psO.tile([P, CHUNK], F32, tag="psO")
                    nj = 4 * ic + 4            # total j-tiles incl. diagonal
                    for jt in range(nj):
                        jc, r = divmod(jt, 4)
                        diag = jc == ic
                        off = P * r if diag else 0      # first query col
                        n = CHUNK - off
                        first, last = jt == 0, jt == nj - 1
                        pss = psS.tile([P, CHUNK], F32, tag="psS")
                        nc.tensor.matmul(
                            pss[:, off:], kT[jc][:, h, P * r:P * (r + 1)],
                            qTi[:, h, off:], start=True, stop=True)
                        at = atp.tile([P, CHUNK], BF16, tag="attnT")
                        nc.scalar.activation(
                            at[:, off:], pss[:, off:],
                            mybir.ActivationFunctionType.Exp, scale=SCALE)
                        if diag:
                            nc.vector.tensor_tensor(
                                at[:, off:off + P], at[:, off:off + P],
                                mask[:], mybir.AluOpType.mult)
                        nc.tensor.matmul(
                            pso[:, off:], vnat[jc][:, r, HD * h:HD * (h + 1)],
                            at[:, off:], start=first, stop=last)
                        nc.tensor.matmul(psd[:, off:], ones[:], at[:, off:],
                                         start=first, stop=last)
                    rec = rcp.tile([P, CHUNK], F32, tag="recip")
                    nc.vector.reciprocal_approx_fast(rec[:], psd[:])
                    nc.vector.tensor_tensor(aout[:, h, :], pso[:], rec[:],
                                            mybir.AluOpType.mult)
                prev_aout = aout

            o_proj(NCHUNK - 1, prev_aout)
    nc.compile()
    return nc


_NC = None


def _get_nc():
    global _NC
    if _NC is None:
        _NC = _build()
    return _NC


def _host_prep(x, Wq_down, Wq_up, Wq_rope, Wkv_down, Wk_up, Wk_rope, Wv_up, Wo):
    """Build the 8 per-core input maps (all host-side layout prep)."""
    # rope tables, replicated to 128 partitions with NeoX sign baked into sin
    half = RD // 2
    inv_freq = 1.0 / (BASE ** (np.arange(half, dtype=np.float64) / half))
    ang = np.arange(S, dtype=np.float64)[None, :] * inv_freq[:, None]  # [32, S]
    cos32 = np.cos(ang)
    sin32 = np.sin(ang)
    cosr = np.tile(cos32, (4, 1)).astype(_BF16)                        # [128,S]
    sinr = np.concatenate([-sin32, sin32, -sin32, sin32], 0).astype(_BF16)

    # causal mask for the 128x128 diagonal block: key p visible to query c
    pidx = np.arange(P)[:, None]
    cidx = np.arange(P)[None, :]
    maskd = (pidx <= cidx).astype(_BF16)

    xT = [np.ascontiguousarray(x[b].T).astype(_BF16) for b in range(B)]
    wqd = Wq_down.astype(_BF16)
    wkvd = Wkv_down.astype(_BF16)

    in_maps = []
    for c in range(8):
        b, g = divmod(c, 4)
        heads = range(HLOC * g, HLOC * (g + 1))
        wqcat = np.empty((QR, HLOC * HD), np.float32)
        for i, h in enumerate(heads):
            wqcat[:, i * HD:i * HD + ND] = Wq_up[:, h * ND:(h + 1) * ND]
            wqcat[:, i * HD + ND:(i + 1) * HD] = Wq_rope[:, h * RD:(h + 1) * RD]
        in_maps.append({
            "xT": xT[b],
            "wqd": wqd,
            "wkvd": wkvd,
            "wkr": np.ascontiguousarray(
                Wk_rope[:, g * HLOC * RD:(g + 1) * HLOC * RD]).astype(_BF16),
            "wqcat": wqcat.astype(_BF16),
            "wkup": np.ascontiguousarray(
                Wk_up[:, g * HLOC * ND:(g + 1) * HLOC * ND]).astype(_BF16),
            "wvup": np.ascontiguousarray(
                Wv_up[:, g * HLOC * HD:(g + 1) * HLOC * HD]).astype(_BF16),
            "wo": np.ascontiguousarray(
                Wo[g * HLOC * HD:(g + 1) * HLOC * HD, :]).astype(_BF16),
            "cosr": cosr,
            "sinr": sinr,
            "maskd": maskd,
        })
    return in_maps


def kernel(x, Wq_down, Wq_up, Wq_rope, Wkv_down, Wk_up, Wk_rope, Wv_up, Wo,
           _trace=False, _trace_kwargs=None):
    x = np.asarray(x, dtype=np.float32)
    args = [np.asarray(a, dtype=np.float32) for a in
            (Wq_down, Wq_up, Wq_rope, Wkv_down, Wk_up, Wk_rope, Wv_up, Wo)]
    in_maps = _host_prep(x, *args)
    nc = _get_nc()
    res = run_bass_kernel_spmd(nc, in_maps, core_ids=list(range(8)),
                               trace=_trace, **(_trace_kwargs or {}))
    kernel._last_results = res
    out = np.zeros((B, S, D), np.float32)
    for c in range(8):
        out[c // 4] += res.results[c]["o_part"].astype(np.float32)
    return out


# revision 19
# speedup vs baseline: 1.3277x; 1.0359x over previous
"""MLA attention (DeepSeek-style) Trainium2 Bass kernel, 8-core SPMD.

Sharding: core c handles batch b = c//4 and head-group g = c%4 (4 of 16 heads).
Down-projections are replicated per batch; up-projections / attention / o-proj
are head-parallel. Host sums the 4 partial o-projections per batch.

v2 schedule (vs baseline): all activations double-buffered so chunk ic+1's
down-projections interleave into chunk ic's attention stalls; rope is
decoupled from PSUM via ScalarE drains to SBUF + bf16 DVE math; kT/vnat are
per-chunk tiles (no false cross-chunk deps); causal diagonal is trimmed at
128 granularity; psS has 3 PSUM banks so the scores->exp->attnout pipeline
runs at exp rate; small weights resident in SBUF; bf16 latents and output
partials.
"""

import numpy as np
import ml_dtypes

import concourse.bacc as bacc
import concourse.mybir as mybir
import concourse.tile as tile
from concourse.bass_utils import run_bass_kernel_spmd

F32 = mybir.dt.float32
BF16 = mybir.dt.bfloat16

B, S, D = 2, 2048, 2048
H, HD = 16, 128
RD, ND = 64, 64
KVR, QR = 512, 1024
BASE = 10000.0
HLOC = 4                 # heads per core
CHUNK = 512
NCHUNK = S // CHUNK      # 4
P = 128
SCALE = HD ** -0.5

_BF16 = ml_dtypes.bfloat16


def _build():
    nc = bacc.Bacc("TRN2", target_bir_lowering=False, debug=False)

    xT = nc.dram_tensor("xT", [D, S], BF16, kind="ExternalInput").ap()
    wqd = nc.dram_tensor("wqd", [D, QR], BF16, kind="ExternalInput").ap()
    wkvd = nc.dram_tensor("wkvd", [D, KVR], BF16, kind="ExternalInput").ap()
    wkr = nc.dram_tensor("wkr", [D, HLOC * RD], BF16, kind="ExternalInput").ap()
    wqcat = nc.dram_tensor("wqcat", [QR, HLOC * HD], BF16, kind="ExternalInput").ap()
    wkup = nc.dram_tensor("wkup", [KVR, HLOC * ND], BF16, kind="ExternalInput").ap()
    wvup = nc.dram_tensor("wvup", [KVR, HLOC * HD], BF16, kind="ExternalInput").ap()
    wo = nc.dram_tensor("wo", [HLOC * HD, D], BF16, kind="ExternalInput").ap()
    cosr = nc.dram_tensor("cosr", [P, S], BF16, kind="ExternalInput").ap()
    sinr = nc.dram_tensor("sinr", [P, S], BF16, kind="ExternalInput").ap()
    maskd = nc.dram_tensor("maskd", [P, P], BF16, kind="ExternalInput").ap()
    o_part = nc.dram_tensor("o_part", [S, D], BF16, kind="ExternalOutput").ap()

    xT_r = xT.rearrange("(dt p) s -> p dt s", p=P)          # [128, 16, S]
    wqd_r = wqd.rearrange("(dt p) q -> p dt q", p=P)        # [128, 16, 1024]
    wkvd_r = wkvd.rearrange("(dt p) q -> p dt q", p=P)      # [128, 16, 512]
    wkr_r = wkr.rearrange("(dt p) q -> p dt q", p=P)        # [128, 16, 256]
    wqcat_r = wqcat.rearrange("(qt p) c -> p qt c", p=P)    # [128, 8, 512]
    wkup_r = wkup.rearrange("(kt p) c -> p kt c", p=P)      # [128, 4, 256]
    wvup_r = wvup.rearrange("(kt p) c -> p kt c", p=P)      # [128, 4, 512]
    wo_r = wo.rearrange("(kt p) d -> p kt d", p=P)          # [128, 4, 2048]
    o_r = o_part.rearrange("(st p) d -> p st d", p=P)       # [128, 16, 2048]

    with tile.TileContext(nc) as tc:
        with (
            tc.tile_pool(name="persist", bufs=1) as pp,
            tc.tile_pool(name="acts", bufs=2) as ap_,
            tc.tile_pool(name="wstream", bufs=3) as wp,
            tc.tile_pool(name="rope", bufs=2) as rp,
            tc.tile_pool(name="attn", bufs=3) as atp,
            tc.tile_pool(name="recp", bufs=2) as rcp,
            tc.tile_pool(name="outp", bufs=2) as op_,
            tc.tile_pool(name="aoutp", bufs=2) as aop,
            tc.tile_pool(name="psA", bufs=2, space="PSUM") as psA,
            tc.tile_pool(name="psS", bufs=3, space="PSUM") as psS,
            tc.tile_pool(name="psD", bufs=1, space="PSUM") as psD,
            tc.tile_pool(name="psO", bufs=2, space="PSUM") as psO,
        ):
            # ---------------- persistent tiles ----------------
            kT = [pp.tile([P, HLOC, CHUNK], BF16, name=f"kT{j}", tag=f"kT{j}")
                  for j in range(NCHUNK)]                     # per-chunk K^T
            vnat = [pp.tile([P, CHUNK // P, HLOC * HD], BF16, name=f"vn{j}", tag=f"vn{j}")
                    for j in range(NCHUNK)]                   # per-chunk V nat
            mask = pp.tile([P, P], BF16, tag="mask")
            ones = pp.tile([P, P], BF16, tag="ones")
            wo_t = pp.tile([P, HLOC, D], BF16, tag="wo")
            wqc_t = pp.tile([P, QR // P, HLOC * HD], BF16, tag="wqc")
            wku_t = pp.tile([P, KVR // P, HLOC * ND], BF16, tag="wku")
            wvu_t = pp.tile([P, KVR // P, HLOC * HD], BF16, tag="wvu")
            wkr_t = pp.tile([P, D // P, HLOC * RD], BF16, tag="wkr")
            cos_t = pp.tile([P, S], BF16, tag="cos")
            sin_t = pp.tile([P, S], BF16, tag="sin")

            nc.vector.memset(ones[:], 1.0)
            # PE warm-up during the initial DMA ramp: ~3.5us of tiny matmuls
            # (only dep: the memset) un-throttle the HAM clock gate before the
            # first real matmul arrives
            wps = psA.tile([P, CHUNK], F32, name="warmps", tag="psA")
            for _ in range(155):
                nc.tensor.matmul(wps[0:64, 0:64], ones[:, 0:64],
                                 ones[:, 0:64], start=True, stop=True)

            def load_residents():
                """Emitted after chunk 0's critical x/weight DMAs so these
                don't compete for HBM bandwidth before the first matmul;
                ordered by first use."""
                nc.sync.dma_start(mask[:], maskd[:])
                nc.sync.dma_start(cos_t[:], cosr[:])
                nc.sync.dma_start(sin_t[:], sinr[:])
                nc.sync.dma_start(wkr_t[:], wkr_r[:])
                nc.sync.dma_start(wqc_t[:], wqcat_r[:])
                nc.sync.dma_start(wku_t[:], wkup_r[:])
                nc.sync.dma_start(wvu_t[:], wvup_r[:])
                for kt_ in range(HLOC):
                    nc.sync.dma_start(wo_t[:, kt_, :], wo_r[:, kt_, :])

            def o_proj(ic, aout, sts=range(CHUNK // P), final=False):
                """Project previous chunk's attention output; PE filler work
                staged across the next chunk's attention loop."""
                for st in sts:
                    osb = op_.tile([P, D], BF16, tag="osb")
                    for dc in range(D // CHUNK):
                        ps = psA.tile([P, CHUNK], F32, tag="psA")
                        for kt_ in range(HLOC):
                            nc.tensor.matmul(
                                ps[:], aout[:, kt_, P * st:P * (st + 1)],
                                wo_t[:, kt_, CHUNK * dc:CHUNK * (dc + 1)],
                                start=(kt_ == 0), stop=(kt_ == HLOC - 1))
                        if final and dc % 2 == 1:
                            # ScalarE is idle after the last exp
                            nc.scalar.copy(
                                osb[:, CHUNK * dc:CHUNK * (dc + 1)], ps[:])
                        else:
                            nc.vector.tensor_copy(
                                osb[:, CHUNK * dc:CHUNK * (dc + 1)], ps[:])
                        # per-dc store, alternated across two DMA rings so
                        # the final output drain runs on both concurrently
                        eng = nc.gpsimd if dc % 2 == 0 else nc.sync
                        eng.dma_start(
                            o_r[:, ic * (CHUNK // P) + st,
                                CHUNK * dc:CHUNK * (dc + 1)],
                            osb[:, CHUNK * dc:CHUNK * (dc + 1)])

            # ---------------- chunk loop ----------------
            for ic in range(NCHUNK):
                sl = slice(ic * CHUNK, (ic + 1) * CHUNK)

                # first weight strip before the bulky x load so the first
                # matmul's operands arrive earliest
                ws0 = wp.tile([P, D // P, 2 * P], BF16, name="ws0",
                              tag="wstrip")
                nc.sync.dma_start(ws0[:, 0:8, :], wqd_r[:, 0:8, 0:2 * P])
                nc.sync.dma_start(ws0[:, 8:16, :], wqd_r[:, 8:16, 0:2 * P])
                xc = ap_.tile([P, D // P, CHUNK], BF16, tag="xc")
                for dq in range(4):
                    nc.sync.dma_start(xc[:, 4 * dq:4 * (dq + 1), :],
                                      xT_r[:, 4 * dq:4 * (dq + 1), sl])
                cos_c = cos_t[:, sl]
                sin_c = sin_t[:, sl]

                # ---- q_latT [1024, CHUNK] (bf16) ----
                qlat = ap_.tile([P, QR // P, CHUNK], BF16, tag="qlat")
                for cp in range(QR // P // 2):          # c-tile pairs
                    if cp == 0:
                        ws = ws0
                    else:
                        ws = wp.tile([P, D // P, 2 * P], BF16, tag="wstrip")
                        nc.sync.dma_start(
                            ws[:], wqd_r[:, :, 2 * P * cp:2 * P * (cp + 1)])
                    for ci in range(2):
                        c = 2 * cp + ci
                        ps = psA.tile([P, CHUNK], F32, tag="psA")
                        for dt_ in range(D // P):
                            nc.tensor.matmul(
                                ps[:], ws[:, dt_, P * ci:P * (ci + 1)],
                                xc[:, dt_, :],
                                start=(dt_ == 0), stop=(dt_ == D // P - 1))
                        nc.scalar.copy(qlat[:, c, :], ps[:])

                # ---- kv_latT [512, CHUNK] (bf16) ----
                kvlat = ap_.tile([P, KVR // P, CHUNK], BF16, tag="kvlat")
                for cp in range(KVR // P // 2):
                    ws = wp.tile([P, D // P, 2 * P], BF16, tag="wstrip")
                    nc.sync.dma_start(
                        ws[:], wkvd_r[:, :, 2 * P * cp:2 * P * (cp + 1)])
                    for ci in range(2):
                        c = 2 * cp + ci
                        ps = psA.tile([P, CHUNK], F32, tag="psA")
                        for dt_ in range(D // P):
                            nc.tensor.matmul(
                                ps[:], ws[:, dt_, P * ci:P * (ci + 1)],
                                xc[:, dt_, :],
                                start=(dt_ == 0), stop=(dt_ == D // P - 1))
                        nc.scalar.copy(kvlat[:, c, :], ps[:])

                if ic == 0:
                    load_residents()

                # ---- k_pe: head pair a -> heads (2a, 2a+1) rope dims ----
                # drain psum via ScalarE to SBUF bf16, rope on DVE from SBUF
                for a in range(2):
                    ps = psA.tile([P, CHUNK], F32, tag="psA")
                    for dt_ in range(D // P):
                        nc.tensor.matmul(
                            ps[:], wkr_t[:, dt_, P * a:P * (a + 1)],
                            xc[:, dt_, :],
                            start=(dt_ == 0), stop=(dt_ == D // P - 1))
                    raw = rp.tile([P, CHUNK], BF16, tag="kraw")
                    sh = rp.tile([P, CHUNK], BF16, tag="ksh")
                    scr = rp.tile([P, CHUNK], BF16, tag="kscr")
                    nc.scalar.copy(raw[:], ps[:])
                    # NeoX rotation: shifted halves within each 64-row block
                    for b in (0, 64):
                        nc.vector.tensor_copy(sh[b:b + 32, :],
                                              raw[b + 32:b + 64, :])
                        nc.vector.tensor_copy(sh[b + 32:b + 64, :],
                                              raw[b:b + 32, :])
                    nc.vector.tensor_tensor(sh[:], sh[:], sin_c,
                                            mybir.AluOpType.mult)
                    nc.vector.tensor_tensor(scr[:], raw[:], cos_c,
                                            mybir.AluOpType.mult)
                    nc.vector.tensor_tensor(kT[ic][64:128, 2 * a, :],
                                            scr[0:64, :], sh[0:64, :],
                                            mybir.AluOpType.add)
                    nc.vector.tensor_tensor(kT[ic][64:128, 2 * a + 1, :],
                                            scr[64:128, :], sh[64:128, :],
                                            mybir.AluOpType.add)

                # ---- q heads: c-tile h = head h [nope64 | pe64] ----
                qTi = ap_.tile([P, HLOC, CHUNK], BF16, tag="qTi")
                for h in range(HLOC):
                    ps = psA.tile([P, CHUNK], F32, tag="psA")
                    for qt in range(QR // P):
                        nc.tensor.matmul(
                            ps[:], wqc_t[:, qt, P * h:P * (h + 1)],
                            qlat[:, qt, :],
                            start=(qt == 0), stop=(qt == QR // P - 1))
                    nc.scalar.copy(qTi[0:64, h, :], ps[0:64, :])
                    raw = rp.tile([P, CHUNK], BF16, tag="qraw")
                    sh = rp.tile([P, CHUNK], BF16, tag="qsh")
                    scr = rp.tile([P, CHUNK], BF16, tag="qscr")
                    nc.scalar.copy(raw[64:128, :], ps[64:128, :])
                    nc.vector.tensor_copy(sh[64:96, :], raw[96:128, :])
                    nc.vector.tensor_copy(sh[96:128, :], raw[64:96, :])
                    nc.vector.tensor_tensor(sh[64:128, :], sh[64:128, :],
                                            sin_c[64:128, :],
                                            mybir.AluOpType.mult)
                    nc.vector.tensor_tensor(scr[64:128, :], raw[64:128, :],
                                            cos_c[64:128, :],
                                            mybir.AluOpType.mult)
                    nc.vector.tensor_tensor(qTi[64:128, h, :],
                                            scr[64:128, :], sh[64:128, :],
                                            mybir.AluOpType.add)

                # ---- k_nope: head pair a -> heads (2a, 2a+1) nope dims ----
                for a in range(2):
                    ps = psA.tile([P, CHUNK], F32, tag="psA")
                    for kt_ in range(KVR // P):
                        nc.tensor.matmul(
                            ps[:], wku_t[:, kt_, P * a:P * (a + 1)],
                            kvlat[:, kt_, :],
                            start=(kt_ == 0), stop=(kt_ == KVR // P - 1))
                    if ic == 0:
                        # chunk 0's window is DVE-bound (rope chain); ScalarE
                        # has slack there (no overlapping attention exp)
                        nc.scalar.copy(kT[ic][0:64, 2 * a, :], ps[0:64, :])
                        nc.scalar.copy(kT[ic][0:64, 2 * a + 1, :],
                                       ps[64:128, :])
                    else:
                        nc.vector.tensor_copy(kT[ic][0:64, 2 * a, :],
                                              ps[0:64, :])
                        nc.vector.tensor_copy(kT[ic][0:64, 2 * a + 1, :],
                                              ps[64:128, :])

                # ---- v natural [CHUNK, 512] ----
                for st in range(CHUNK // P):
                    ps = psA.tile([P, HLOC * HD], F32, tag="psA")
                    for kt_ in range(KVR // P):
                        nc.tensor.matmul(
                            ps[:], kvlat[:, kt_, P * st:P * (st + 1)],
                            wvu_t[:, kt_, :],
                            start=(kt_ == 0), stop=(kt_ == KVR // P - 1))
                    if ic == 0:
                        nc.scalar.copy(vnat[ic][:, st, :], ps[:])
                    else:
                        nc.vector.tensor_copy(vnat[ic][:, st, :], ps[:])

                # ---- o-projection of the PREVIOUS chunk: PE filler that
                # covers this chunk's rope/DVE latency before attention
                if ic > 0:
                    o_proj(ic - 1, prev_aout, sts=(0, 1))

                # ---- attention for this query chunk (diagonal trimmed) ----
                aout = aop.tile([P, HLOC, CHUNK], BF16, tag="aout")
                for h in range(HLOC):
                    if ic > 0 and h in (2, 3):
                        # reserve late PE filler for the exp-paced tail
                        o_proj(ic - 1, prev_aout, sts=(h,))
                    psd = psD.tile([P, CHUNK], F32, tag="psD")
                    pso = psO.tile([P, CHUNK], F32, tag="psO")
                    nj = 4 * ic + 4            # total j-tiles incl. diagonal
                    for jt in range(nj):
                        jc, r = divmod(jt, 4)
                        diag = jc == ic
                        off = P * r if diag else 0      # first query col
                        n = CHUNK - off
                        first, last = jt == 0, jt == nj - 1
                        pss = psS.tile([P, CHUNK], F32, tag="psS")
                        nc.tensor.matmul(
                            pss[:, off:], kT[jc][:, h, P * r:P * (r + 1)],
                            qTi[:, h, off:], start=True, stop=True)
                        at = atp.tile([P, CHUNK], BF16, tag="attnT")
                        nc.scalar.activation(
                            at[:, off:], pss[:, off:],
                            mybir.ActivationFunctionType.Exp, scale=SCALE)
                        if diag:
                            nc.vector.tensor_tensor(
                                at[:, off:off + P], at[:, off:off + P],
                                mask[:], mybir.AluOpType.mult)
                        nc.tensor.matmul(
                            pso[:, off:], vnat[jc][:, r, HD * h:HD * (h + 1)],
                            at[:, off:], start=first, stop=last)
                        nc.tensor.matmul(psd[:, off:], ones[:], at[:, off:],
                                         start=first, stop=last)
                    rec = rcp.tile([P, CHUNK], F32, tag="recip")
                    nc.vector.reciprocal_approx_fast(rec[:], psd[:])
                    if ic == NCHUNK - 1 and h == HLOC - 1:
                        # final head gates o_proj(3): normalize per query
                        # slice so its st-blocks start earlier
                        for stq in range(CHUNK // P):
                            qs = slice(P * stq, P * (stq + 1))
                            nc.vector.tensor_tensor(
                                aout[:, h, qs], pso[:, qs], rec[:, qs],
                                mybir.AluOpType.mult)
                    else:
                        nc.vector.tensor_tensor(aout[:, h, :], pso[:], rec[:],
                                                mybir.AluOpType.mult)
                prev_aout = aout

            o_proj(NCHUNK - 1, prev_aout, final=True)
    nc.compile()
    return nc


_NC = None


def _get_nc():
    global _NC
    if _NC is None:
        _NC = _build()
    return _NC


def _host_prep(x, Wq_down, Wq_up, Wq_rope, Wkv_down, Wk_up, Wk_rope, Wv_up, Wo):
    """Build the 8 per-core input maps (all host-side layout prep)."""
    # rope tables, replicated to 128 partitions with NeoX sign baked into sin
    half = RD // 2
    inv_freq = 1.0 / (BASE ** (np.arange(half, dtype=np.float64) / half))
    ang = np.arange(S, dtype=np.float64)[None, :] * inv_freq[:, None]  # [32, S]
    cos32 = np.cos(ang)
    sin32 = np.sin(ang)
    cosr = np.tile(cos32, (4, 1)).astype(_BF16)                        # [128,S]
    sinr = np.concatenate([-sin32, sin32, -sin32, sin32], 0).astype(_BF16)

    # causal mask for the 128x128 diagonal block: key p visible to query c
    pidx = np.arange(P)[:, None]
    cidx = np.arange(P)[None, :]
    maskd = (pidx <= cidx).astype(_BF16)

    xT = [np.ascontiguousarray(x[b].T).astype(_BF16) for b in range(B)]
    wqd = Wq_down.astype(_BF16)
    wkvd = Wkv_down.astype(_BF16)

    in_maps = []
    for c in range(8):
        b, g = divmod(c, 4)
        heads = range(HLOC * g, HLOC * (g + 1))
        wqcat = np.empty((QR, HLOC * HD), np.float32)
        for i, h in enumerate(heads):
            wqcat[:, i * HD:i * HD + ND] = Wq_up[:, h * ND:(h + 1) * ND]
            wqcat[:, i * HD + ND:(i + 1) * HD] = Wq_rope[:, h * RD:(h + 1) * RD]
        in_maps.append({
            "xT": xT[b],
            "wqd": wqd,
            "wkvd": wkvd,
            "wkr": np.ascontiguousarray(
                Wk_rope[:, g * HLOC * RD:(g + 1) * HLOC * RD]).astype(_BF16),
            "wqcat": wqcat.astype(_BF16),
            "wkup": np.ascontiguousarray(
                Wk_up[:, g * HLOC * ND:(g + 1) * HLOC * ND]).astype(_BF16),
            "wvup": np.ascontiguousarray(
                Wv_up[:, g * HLOC * HD:(g + 1) * HLOC * HD]).astype(_BF16),
            "wo": np.ascontiguousarray(
                Wo[g * HLOC * HD:(g + 1) * HLOC * HD, :]).astype(_BF16),
            "cosr": cosr,
            "sinr": sinr,
            "maskd": maskd,
        })
    return in_maps


def kernel(x, Wq_down, Wq_up, Wq_rope, Wkv_down, Wk_up, Wk_rope, Wv_up, Wo,
           _trace=False, _trace_kwargs=None):
    x = np.asarray(x, dtype=np.float32)
    args = [np.asarray(a, dtype=np.float32) for a in
            (Wq_down, Wq_up, Wq_rope, Wkv_down, Wk_up, Wk_rope, Wv_up, Wo)]
    in_maps = _host_prep(x, *args)
    nc = _get_nc()
    res = run_bass_kernel_spmd(nc, in_maps, core_ids=list(range(8)),
                               trace=_trace, **(_trace_kwargs or {}))
    kernel._last_results = res
    out = np.zeros((B, S, D), np.float32)
    for c in range(8):
        out[c // 4] += res.results[c]["o_part"].astype(np.float32)
    return out
